# revision 39
# baseline (speedup 1.0000x reference)
"""MultiHeadAttention (B=2, S=2048, D=1024, H=16) on 8 Trainium2 NeuronCores.

v2 sharding (tensor-parallel heads + AllGather, per the sharding hint):
core i = (b, g) with b = i//4 (batch), g = i%4 (rank in the 4-core batch group).
  - Q/K/V projections: core computes only its 4 heads (256 of 1024 output dims)
    over the whole batch (2048 rows). Wq/Wk/Wv column slices are fed as data.
  - Attention: its 4 heads, all 2048 queries, processed in two q-halves.
  - After each q-half, the normalised context slab [256, 1024] is AllGathered
    within the batch group -> full ctx.T [1024, 1024] per half.
  - Output projection: core computes a disjoint 256-column slice of the final
    output (Wo column slice fed as data) over ALL 2048 rows; host concatenates
    feature slices. This keeps the SPMD program uniform across cores.

Layouts are transposed (feature-major) so no on-device transposes are needed:
  Q.T/K.T = (W.T).T @ X.T   [o, s]  (o on partitions)
  V       = (X.T).T @ Wv.T  [s, o]  + ones column (gives softmax denominators)
  S.T     = K_h @ Q_h.T     [s, q]  chunks; exp on ACT with fused 1/8 scale
  ctx.T_h = ([V_h | 1]).T @ P.T -> [65, q]; row 64 = softmax denominator
  out.T   = (Wo.T slice).T @ ctx.T + bo
Softmax division: reciprocal of row 64 on DVE, replicated across partitions
with a rank-1 matmul (ones x recip), one DVE multiply.
"""

import sys

for _p in ("/opt/trn_rl_repo", "/root/.axon_site/_ro/trn_rl_repo"):
    if _p not in sys.path:
        sys.path.append(_p)

import ml_dtypes
import numpy as np

import concourse.bass as bass
import concourse.tile as tile
from concourse import bacc, mybir
from concourse.bass import ds, ts
from concourse.bass_utils import run_bass_kernel_spmd

P = 128
D = 1024            # d_model
KC = D // P         # 8 contraction chunks
S = 2048            # sequence length per batch
HL = 4              # heads per core
OL = HL * 64        # local output dims for q/k/v projections (256)
JL = OL // P        # local o-chunks (2)
H = 16
DH = 64
NT = S // P         # 16 kv chunks
QH = S // 2         # q half (1024)

BF = mybir.dt.bfloat16
F32 = mybir.dt.float32
ID = mybir.ActivationFunctionType.Identity
EXP = mybir.ActivationFunctionType.Exp

_CACHE = {}


def _declare_io(nc):
    xqT = nc.dram_tensor("xqT", [D, S], BF, kind="ExternalInput").ap()
    xkT = nc.dram_tensor("xkT", [D, S], BF, kind="ExternalInput").ap()
    xvT = nc.dram_tensor("xvT", [D, S], BF, kind="ExternalInput").ap()
    wqT = nc.dram_tensor("wqT", [D, OL], BF, kind="ExternalInput").ap()
    wkT = nc.dram_tensor("wkT", [D, OL], BF, kind="ExternalInput").ap()
    wvT = nc.dram_tensor("wvT", [D, OL], BF, kind="ExternalInput").ap()
    woT = nc.dram_tensor("woT", [OL, D], BF, kind="ExternalInput").ap()
    bq = nc.dram_tensor("bq", [OL], F32, kind="ExternalInput").ap()
    bk = nc.dram_tensor("bk", [OL], F32, kind="ExternalInput").ap()
    bo = nc.dram_tensor("bo", [D], F32, kind="ExternalInput").ap()
    bvb = nc.dram_tensor("bvb", [1, OL], BF, kind="ExternalInput").ap()
    outT = nc.dram_tensor("outT", [OL, S], F32, kind="ExternalOutput").ap()
    return dict(xqT=xqT, xkT=xkT, xvT=xvT, wqT=wqT, wkT=wkT, wvT=wvT, woT=woT,
                bq=bq, bk=bk, bo=bo, bvb=bvb, outT=outT)


def _emit_body(nc, tc, ctx, io, rep=0, collectives=True, stop_after=None):
    from contextlib import ExitStack
    xqT, xkT, xvT = io["xqT"], io["xkT"], io["xvT"]
    wqT, wkT, wvT, woT = io["wqT"], io["wkT"], io["wvT"], io["woT"]
    bq, bk, bo, bvb, outT = io["bq"], io["bk"], io["bo"], io["bvb"], io["outT"]
    if True:
        const = ctx.enter_context(tc.tile_pool(name="const", bufs=1))
        wpool = ctx.enter_context(tc.tile_pool(name="w", bufs=1))
        big = ctx.enter_context(tc.tile_pool(name="big", bufs=1))
        xpool = ctx.enter_context(tc.tile_pool(name="xf", bufs=2))
        stage = ctx.enter_context(tc.tile_pool(name="stage", bufs=5))
        ptpool = ctx.enter_context(tc.tile_pool(name="pt", bufs=6))
        small = ctx.enter_context(tc.tile_pool(name="small", bufs=2))
        pbig = ctx.enter_context(tc.tile_pool(name="pbig", bufs=2, space="PSUM"))
        pct = ctx.enter_context(tc.tile_pool(name="pct", bufs=2, space="PSUM"))
        dram = ctx.enter_context(tc.tile_pool(name="dram", bufs=1, space="DRAM"))

        # ---- constants / weights -------------------------------------------
        wq = wpool.tile([P, KC, OL], BF)
        nc.sync.dma_start(wq[:], wqT.rearrange("(c p) o -> p c o", p=P))
        wk = wpool.tile([P, KC, OL], BF)
        nc.sync.dma_start(wk[:], wkT.rearrange("(c p) o -> p c o", p=P))
        wv = wpool.tile([P, KC, OL], BF)
        wo = wpool.tile([P, JL, D], BF)

        bq_sb = const.tile([P, JL], F32)
        nc.sync.dma_start(bq_sb[:], bq.rearrange("(c p) -> p c", p=P))
        bk_sb = const.tile([P, JL], F32)
        nc.sync.dma_start(bk_sb[:], bk.rearrange("(c p) -> p c", p=P))
        bo_sb = const.tile([P, KC], F32)
        nc.sync.dma_start(bo_sb[:], bo.rearrange("(c p) -> p c", p=P))
        bvb_sb = const.tile([1, OL], BF)

        ones64 = const.tile([1, DH], BF)
        nc.vector.memset(ones64[:], 1.0)
        ones128 = const.tile([1, P], BF)
        nc.vector.memset(ones128[:], 1.0)

        # ---- Q and K projections (transposed layout [o_local, s]) ----------
        qt = big.tile([P, JL, S], BF)
        kt = big.tile([P, JL, S], BF)
        for name, src, w_sb, b_sb, dst in (
            ("q", xqT, wq, bq_sb, qt),
            ("k", xkT, wk, bk_sb, kt),
        ):
            xf = xpool.tile([P, KC, S], BF, tag="xfull")
            src_r = src.rearrange("(c p) r -> p c r", p=P)
            for c in range(KC):
                nc.sync.dma_start(xf[:, c, :], src_r[:, c, :])
            for j in range(JL):
                ps0 = pbig.tile([P, QH], F32, tag="big")
                ps1 = pbig.tile([P, QH], F32, tag="big")
                for c in range(KC):
                    st_, sp_ = (c == 0), (c == KC - 1)
                    lhs = w_sb[:, c, ts(j, P)]
                    nc.tensor.matmul(ps0[:, 0:512], lhs, xf[:, c, 0:512], start=st_, stop=sp_)
                    nc.tensor.matmul(ps0[:, 512:1024], lhs, xf[:, c, 512:1024], start=st_, stop=sp_)
                    nc.tensor.matmul(ps1[:, 0:512], lhs, xf[:, c, 1024:1536], start=st_, stop=sp_)
                    nc.tensor.matmul(ps1[:, 512:1024], lhs, xf[:, c, 1536:2048], start=st_, stop=sp_)
                nc.vector.tensor_scalar_add(dst[:, j, 0:QH], ps0[:], b_sb[:, j : j + 1])
                nc.vector.tensor_scalar_add(dst[:, j, QH:S], ps1[:], b_sb[:, j : j + 1])

        if stop_after == "qk":
            return
        # ---- V projection (emitted per chunk, interleaved into attention) ---
        nc.sync.dma_start(wv[:], wvT.rearrange("(c p) o -> p c o", p=P))
        nc.sync.dma_start(wo[:], woT.rearrange("(c p) o -> p c o", p=P))
        nc.sync.dma_start(bvb_sb[:], bvb[:])
        v = big.tile([P, NT, HL, DH + 1], BF)
        nc.vector.memset(v[:, :, :, DH : DH + 1], 1.0)
        xvf = xpool.tile([P, KC, S], BF, tag="xfull")
        xvT_r = xvT.rearrange("(c p) r -> p c r", p=P)
        for c in range(KC):
            nc.sync.dma_start(xvf[:, c, :], xvT_r[:, c, :])

        def v_proj_chunk(r):
            ps = pbig.tile([P, QH], F32, tag="big")
            for c in range(KC):
                nc.tensor.matmul(
                    ps[:, 0:OL], xvf[:, c, ts(r, P)], wv[:, c, :],
                    start=(c == 0), stop=False,
                )
            nc.tensor.matmul(ps[:, 0:OL], ones128[:], bvb_sb[:], start=False, stop=True)
            nc.vector.tensor_copy(
                v[:, r, :, 0:DH], ps[:, 0:OL].rearrange("p (h d) -> p h d", d=DH)
            )

        for r in range(NT):
            v_proj_chunk(r)

        if stop_after == "v":
            return
        # ---- attention + per-half AllGather + out-projection ----------------
        ctl = big.tile([P, JL, S], BF)   # local ctx.T (4 heads), normalised
        outT_r = outT.rearrange("(j p) q -> p j q", p=P)

        # Attention over q blocks. Block (q0, w=1024): per head-pair, two
        # [128, 1024] score tiles (row-group-packed K=64 matmuls, heads at
        # partition offsets 0/64). Block (q0, w=512): ONE [128, 1024] score
        # tile holds BOTH heads' 512 columns so exp stays N=1024.
        def attn_block(q0, w):
            paired = w == 512
            for hp in range(HL // 2):
                j = hp
                ct0 = pct.tile([DH + 1, w], F32, tag="ct")
                ct1 = pct.tile([DH + 1, w], F32, tag="ct")
                for t in range(NT):
                    lhs_k0 = kt[0:DH, j, ts(t, P)]
                    lhs_k1 = kt[DH:P, j, ts(t, P)]
                    if paired:
                        st0 = pbig.tile([P, QH], F32, tag="big")
                        qsl = ds(q0, 512)
                        nc.tensor.matmul(st0[:, 0:512], lhs_k0, qt[0:DH, j, qsl],
                                         start=True, stop=True)
                        nc.tensor.matmul(st0[:, 512:1024], lhs_k1, qt[DH:P, j, qsl],
                                         start=True, stop=True)
                        pt0 = ptpool.tile([P, QH], BF, tag="pt")
                        nc.scalar.activation(pt0[:], st0[:], EXP, scale=0.125)
                        pt_h0, pt_h1 = pt0[:, 0:512], pt0[:, 512:1024]
                    else:
                        st0 = pbig.tile([P, QH], F32, tag="big")
                        st1 = pbig.tile([P, QH], F32, tag="big")
                        for sl in (slice(0, 512), slice(512, 1024)):
                            qsl = ds(q0 + sl.start, 512)
                            nc.tensor.matmul(st0[:, sl], lhs_k0, qt[0:DH, j, qsl],
                                             start=True, stop=True)
                            nc.tensor.matmul(st1[:, sl], lhs_k1, qt[DH:P, j, qsl],
                                             start=True, stop=True)
                        pt0 = ptpool.tile([P, QH], BF, tag="pt")
                        nc.scalar.activation(pt0[:], st0[:], EXP, scale=0.125)
                        pt1 = ptpool.tile([P, QH], BF, tag="pt")
                        nc.scalar.activation(pt1[:], st1[:], EXP, scale=0.125)
                        pt_h0, pt_h1 = pt0, pt1
                    st_, sp_ = (t == 0), (t == NT - 1)
                    lhs_v0 = v[:, t, 2 * hp, :]
                    lhs_v1 = v[:, t, 2 * hp + 1, :]
                    for s0 in range(0, w, 512):
                        nc.tensor.matmul(ct0[:, s0 : s0 + 512], lhs_v0,
                                         pt_h0[:, s0 : s0 + 512], start=st_, stop=sp_)
                        nc.tensor.matmul(ct1[:, s0 : s0 + 512], lhs_v1,
                                         pt_h1[:, s0 : s0 + 512], start=st_, stop=sp_)
                for po, ct in ((0, ct0), (DH, ct1)):
                    rt = small.tile([1, QH], F32, tag="rt")
                    nc.vector.reciprocal(rt[:, 0:w], ct[DH : DH + 1, :])
                    rtb = small.tile([1, QH], BF, tag="rtb")
                    nc.vector.tensor_copy(rtb[:, 0:w], rt[:, 0:w])
                    rep = pbig.tile([P, QH], F32, tag="big")
                    for s0 in range(0, w, 512):
                        nc.tensor.matmul(rep[0:DH, s0 : s0 + 512], ones64[:],
                                         rtb[:, s0 : s0 + 512], start=True, stop=True)
                    rep_sb = small.tile([DH, QH], BF, tag="rep_sb")
                    nc.vector.tensor_copy(rep_sb[:, 0:w], rep[0:DH, 0:w])
                    nc.vector.tensor_tensor(
                        ctl[po : po + DH, j, ds(q0, w)], ct[0:DH, :], rep_sb[:, 0:w],
                        mybir.AluOpType.mult,
                    )

        # Row-parallel out-projection: partial.T[o, q] over the LOCAL 256 ctx
        # dims only (wo holds the matching Wo.T row slice), then a fp32
        # ReduceScatter sums the 4 partials and hands each core exactly its
        # 256 output-feature rows. bo arrives pre-scaled by 1/4 so the
        # reduced sum carries the bias exactly once.
        def partial_out_rs(name, q0, w):
            prs_in = dram.tile([D, w], F32, name=f"prsin{name}_{rep}")
            prs_in_r = prs_in.rearrange("(j p) q -> p j q", p=P)
            for j in range(KC):
                ps = pct.tile([P, QH], F32, tag="ct")
                for cl in range(JL):
                    st_, sp_ = (cl == 0), (cl == JL - 1)
                    lhs = wo[:, cl, ts(j, P)]
                    for b0 in range(0, w, 512):
                        nc.tensor.matmul(ps[:, b0 : b0 + 512], lhs,
                                         ctl[:, cl, ds(q0 + b0, 512)],
                                         start=st_, stop=sp_)
                osb = stage.tile([P, QH], F32, tag="osb")
                nc.vector.tensor_scalar_add(osb[:, 0:w], ps[:, 0:w], bo_sb[:, j : j + 1])
                nc.sync.dma_start(prs_in_r[:, j, :], osb[:, 0:w])
            prs_out = dram.tile([OL, w], F32, name=f"prsout{name}_{rep}")
            if collectives:
                nc.gpsimd.collective_compute(
                    "ReduceScatter",
                    mybir.AluOpType.add,
                    replica_groups=[[0, 1, 2, 3], [4, 5, 6, 7]],
                    ins=[prs_in.opt()],
                    outs=[prs_out.opt()],
                )
            return prs_out


        # two q-halves; each half's partial out-projection + ReduceScatter is
        # emitted right after its attention so the reduce overlaps later work.
        attn_block(0, QH)
        if stop_after == "attn0":
            return
        prs_a = partial_out_rs("a", 0, QH)
        if stop_after == "cc0":
            return
        attn_block(QH, QH)
        if stop_after == "attn1":
            return
        prs_b = partial_out_rs("b", QH, QH)
        if stop_after == "cc1":
            return
        # copy the reduced, bias-included slabs to the output
        for prs, q0 in ((prs_a, 0), (prs_b, QH)):
            prs_r = prs.rearrange("(j p) q -> p j q", p=P)
            for j in range(JL):
                nc.sync.dma_start(outT_r[:, j, ds(q0, QH)], prs_r[:, j, :])

    return


def _build():
    from contextlib import ExitStack

    nc = bacc.Bacc("TRN2", target_bir_lowering=False, debug=False, num_devices=8)
    io = _declare_io(nc)
    with tile.TileContext(nc) as tc, ExitStack() as ctx:
        _emit_body(nc, tc, ctx, io)
    nc.compile()
    return nc


def _get_nc():
    if "nc" not in _CACHE:
        _CACHE["nc"] = _build()
    return _CACHE["nc"]


def make_in_maps(query, key, value, Wq, bq, Wk, bk, Wv, bv, Wo, bo):
    B = 2
    bf16 = ml_dtypes.bfloat16

    query = np.asarray(query, np.float32)
    key = np.asarray(key, np.float32)
    value = np.asarray(value, np.float32)
    wqT = np.ascontiguousarray(np.asarray(Wq, np.float32).T).astype(bf16)
    wkT = np.ascontiguousarray(np.asarray(Wk, np.float32).T).astype(bf16)
    wvT = np.ascontiguousarray(np.asarray(Wv, np.float32).T).astype(bf16)
    woT = np.ascontiguousarray(np.asarray(Wo, np.float32).T).astype(bf16)
    bq = np.asarray(bq, np.float32)
    bk = np.asarray(bk, np.float32)
    bv = np.asarray(bv, np.float32)
    bo = np.asarray(bo, np.float32)
    bvb = bv.astype(bf16).reshape(1, D)
    bo_quarter = (bo * 0.25).astype(np.float32)

    keyT = [np.ascontiguousarray(key[b].T).astype(bf16) for b in range(B)]
    valT = [np.ascontiguousarray(value[b].T).astype(bf16) for b in range(B)]
    qryT = [np.ascontiguousarray(query[b].T).astype(bf16) for b in range(B)]

    in_maps = []
    for i in range(8):
        b, g = i // 4, i % 4
        osl = ds_ = slice(OL * g, OL * (g + 1))
        in_maps.append(
            {
                "xqT": qryT[b],
                "xkT": keyT[b],
                "xvT": valT[b],
                "wqT": np.ascontiguousarray(wqT[:, osl]),
                "wkT": np.ascontiguousarray(wkT[:, osl]),
                "wvT": np.ascontiguousarray(wvT[:, osl]),
                "woT": np.ascontiguousarray(woT[osl, :]),
                "bq": np.ascontiguousarray(bq[osl]),
                "bk": np.ascontiguousarray(bk[osl]),
                "bo": bo_quarter,
                "bvb": np.ascontiguousarray(bvb[:, osl]),
            }
        )
    return in_maps


def kernel(query, key, value, Wq, bq, Wk, bk, Wv, bv, Wo, bo) -> np.ndarray:
    B = 2
    in_maps = make_in_maps(query, key, value, Wq, bq, Wk, bk, Wv, bv, Wo, bo)
    nc = _get_nc()
    res = run_bass_kernel_spmd(nc, in_maps, core_ids=list(range(8)))

    out = np.empty((B, S, D), np.float32)
    for i in range(8):
        b, g = i // 4, i % 4
        out[b, :, OL * g : OL * (g + 1)] = res.results[i]["outT"].T
    return out
